# revision 50
# baseline (speedup 1.0000x reference)
"""GraphSAGE (3x SAGEConv-mean + BN + LeakyReLU) + AvgPool + MLP head on 8
Trainium2 NeuronCores via Bass/Tile.

Sharding: nodes partitioned contiguously across 8 cores (2048 each); weights
replicated; BatchNorm statistics and per-graph pooled sums all-reduced;
per-layer activations all-gathered (node-major fp8 in HBM) so each core can
gather the source rows of its incident edges.

Perf design (vs the bf16 baseline, ~2.8x faster per the cost model):
- Dense self+neigh matmuls run as fp8e4m3 DoubleRow (256 contraction rows
  per pass at 0.5 cycles/row). Self weights use a hi/lo split for near-bf16
  precision; neighbor weights and 1/deg are single-fp8 (their quantization
  noise decorrelates across nodes and is averaged out by the graph pooling).
  Weights are pre-scaled x64 on the host; BatchNorm's scale invariance
  cancels the factor exactly.
- Aggregation computes m^T directly (gathered rows as the stationary
  operand, one-hot S^T with 1/deg folded in as the moving operand), so no
  transposes and no HBM round trip for m. Groups 8-15 of each layer are
  fetched and matmul'd between dense pass-A iterations; dense runs as two
  chunk-half passes so it can start after only half the groups.
- Activations are stored fp8 (halves gather traffic); rst stays bf16 in
  SBUF; no HBM round trips for rst or m.
- BN statistics come from bn_stats/bn_aggr on the PSUM tiles (one DVE pass);
  BN affine + LeakyReLU is a single scalar-engine Lrelu pass (fp8 out).
- Head weights are staged into the dead m8/Gt/wcol/m1T/y8 buffers during
  bn3 so the MLP head is never DMA-bound.
"""

import os
import math

import numpy as np
import ml_dtypes

BF = ml_dtypes.bfloat16
F8 = ml_dtypes.float8_e4m3
C = 8          # cores
P = 128        # partitions
EPS = 1e-5
SLOPE = 0.01
WSCALE = 64.0  # host-side weight pre-scale for fp8 (cancelled by BN)


# --------------------------------------------------------------------------
# Host-side preprocessing (index manipulation + dtype casts / layout only)
# --------------------------------------------------------------------------

def _w8_pack(W):
    """[2048, 2048] -> hi/lo fp8 [128, FT, FT2, 2, 2, 128]:
    w8[p, fo, kt2, hl, ab, m] = q(64*W[(2*kt2+ab)*128+p, fo*128+m])."""
    Ki, Mo = W.shape
    FT = Mo // P
    KT = Ki // P
    KT2 = KT // 2
    Ws = (W * WSCALE).astype(np.float32)
    # [KT2, 2, P(k), FT, P(m)]
    Wr = Ws.reshape(KT2, 2, P, FT, P)
    hi = Wr.astype(F8)
    lo = (Wr - hi.astype(np.float32)).astype(F8)
    out = np.empty((P, FT, KT2, 2, 2, P), F8)
    out[:, :, :, 0] = hi.transpose(2, 3, 0, 1, 4)
    out[:, :, :, 1] = lo.transpose(2, 3, 0, 1, 4)
    return np.ascontiguousarray(out)


def _w8_pack_single(W):
    """[2048, 2048] -> single fp8 [128, FT, FT2, 2, 128] (no lo half)."""
    Ki, Mo = W.shape
    FT = Mo // P
    KT2 = Ki // P // 2
    Wr = (W * WSCALE).astype(np.float32).reshape(KT2, 2, P, FT, P)
    return np.ascontiguousarray(Wr.astype(F8).transpose(2, 3, 0, 1, 4))


def _tile_w(W):
    """[Kin, Mout] -> [128, Mout/128, Kin/128, 128] bf16 (lhsT col tiles)."""
    Ki, Mo = W.shape
    return np.ascontiguousarray(
        W.reshape(Ki // P, P, Mo // P, P).transpose(1, 2, 0, 3)
    ).astype(BF)


def _strip(v, ft):
    """[D] -> [128, D/128] fp32 with [p, t] = v[t*128+p]."""
    return np.ascontiguousarray(v.reshape(ft, P).T).astype(np.float32)


def _preprocess(inputs, G=64):
    h = np.asarray(inputs["h"], np.float32)
    src = np.asarray(inputs["src"], np.int64)
    dst = np.asarray(inputs["dst"], np.int64)
    graph_id = np.asarray(inputs["graph_id"], np.int64)
    N, IN_F = h.shape
    HID = np.asarray(inputs["Ws1"]).shape[1]
    MID = np.asarray(inputs["fc2_w"]).shape[1]
    NCLS = np.asarray(inputs["fc3_w"]).shape[1]
    Nc = N // C
    NG = Nc // P          # dst groups (of 128 nodes) per core
    FT = HID // P
    MT = MID // P

    # --- per-core edge partition, sorted by dst, grouped by 128-node groups
    per_core = []
    gmax = np.ones(NG, np.int64)
    for c in range(C):
        lo = c * Nc
        m = (dst >= lo) & (dst < lo + Nc)
        es = src[m]
        ed = dst[m] - lo
        order = np.argsort(ed, kind="stable")
        es, ed = es[order], ed[order]
        gcnt = np.bincount(ed // P, minlength=NG)
        gmax = np.maximum(gmax, gcnt)
        per_core.append((es, ed, gcnt))
    Kg = [int(x) for x in (gmax + P - 1) // P]   # chunks per group (all cores)
    K = max(Kg)
    EG = K * P                         # padded edge slots per group
    IDXW = EG // 16

    # --- gather indices + S^T matrices (invdeg folded, fp8 hi/lo + bf16)
    idx_all, s1_all, sh_all, pmat_all = [], [], [], []
    for c in range(C):
        es, ed, gcnt = per_core[c]
        deg = np.bincount(ed, minlength=Nc).astype(np.float64)
        invdeg = (1.0 / np.maximum(deg, 1.0)).astype(np.float32)
        iv_hi = invdeg.astype(F8)
        gstart = np.concatenate([[0], np.cumsum(gcnt)])
        idx16 = np.zeros((16, NG, IDXW), np.int16)
        s1 = np.zeros((P, NG, K, P), np.float32)   # [e%128, g, e//128, dst]
        stc = np.zeros((P, NG, K, P), F8)          # invdeg single-fp8
        for g in range(NG):
            seg_s = es[gstart[g]:gstart[g + 1]]
            seg_d = ed[gstart[g]:gstart[g + 1]] - g * P
            n = len(seg_s)
            j = np.arange(n)
            idx16[j % 16, g, j // 16] = seg_s.astype(np.int16)
            dglob = seg_d + g * P
            s1[j % P, g, j // P, seg_d] = invdeg[dglob]
            stc[j % P, g, j // P, seg_d] = iv_hi[dglob]
        idx_all.append(np.tile(idx16, (8, 1, 1)))     # replicate for Q7 cores
        s1_all.append(np.ascontiguousarray(s1.astype(BF)))
        sh_all.append(np.ascontiguousarray(stc))

        gid = graph_id[c * Nc:(c + 1) * Nc]
        pm = np.zeros((Nc, G), np.float32)
        pm[np.arange(Nc), gid] = 1.0
        pmat_all.append(
            np.ascontiguousarray(pm.reshape(NG, P, G).transpose(1, 0, 2)).astype(F8)
        )

    cnt = np.bincount(graph_id, minlength=G).astype(np.float64)
    invcnt = (1.0 / np.maximum(cnt, 1.0)).astype(np.float32)[:, None]

    # --- feature tensors
    h128 = np.zeros((N, P), np.float32)
    h128[:, :IN_F] = h
    h128 = h128.astype(BF)
    hT_all = []
    for c in range(C):
        ht = np.zeros((64, Nc), np.float32)
        ht[:IN_F] = h[c * Nc:(c + 1) * Nc].T
        hT_all.append(ht.astype(BF))

    def pad1(W):          # [IN_F, HID] -> [64, FT, 128]
        Wp = np.zeros((64, HID), np.float32)
        Wp[:IN_F] = W
        return np.ascontiguousarray(Wp.reshape(64, FT, P)).astype(BF)

    shared = {
        "h128": h128,
        "w1s": pad1(np.asarray(inputs["Ws1"], np.float32)),
        "w1n": pad1(np.asarray(inputs["Wn1"], np.float32)),
        "w2s": _w8_pack(np.asarray(inputs["Ws2"], np.float32)),
        "w2n": _w8_pack_single(np.asarray(inputs["Wn2"], np.float32)),
        "w3s": _w8_pack(np.asarray(inputs["Ws3"], np.float32)),
        "w3n": _w8_pack_single(np.asarray(inputs["Wn3"], np.float32)),
        "wf1": _tile_w(np.asarray(inputs["fc1_w"], np.float32)),
        "wf2": _tile_w(np.asarray(inputs["fc2_w"], np.float32)),
        "wf3": np.ascontiguousarray(
            np.asarray(inputs["fc3_w"], np.float32).reshape(MT, P, NCLS)
            .transpose(1, 0, 2)).astype(BF),
        "bn1g": _strip(np.asarray(inputs["g1"], np.float32), FT),
        "bn1b": _strip(np.asarray(inputs["be1"], np.float32), FT),
        "bn2g": _strip(np.asarray(inputs["g2"], np.float32), FT),
        "bn2b": _strip(np.asarray(inputs["be2"], np.float32), FT),
        "bn3g": _strip(np.asarray(inputs["g3"], np.float32), FT),
        "bn3b": _strip(np.asarray(inputs["be3"], np.float32), FT),
        "f1b": _strip(np.asarray(inputs["fc1_b"], np.float32), FT),
        "f2b": _strip(np.asarray(inputs["fc2_b"], np.float32), MT),
        "f3b": np.asarray(inputs["fc3_b"], np.float32)[:, None].copy(),
        "invcnt": invcnt,
        "chain": np.zeros((G, NCLS), np.float32),
    }
    in_maps = []
    for c in range(C):
        m = dict(shared)
        m.update({
            "hT": hT_all[c],
            "gidx": idx_all[c],
            "s1": s1_all[c],
            "stc": sh_all[c],
            "pmat": pmat_all[c],
        })
        in_maps.append(m)

    meta = dict(N=N, Nc=Nc, NG=NG, FT=FT, MT=MT, HID=HID, MID=MID, NCLS=NCLS,
                K=K, EG=EG, IDXW=IDXW, G=G, Kg=Kg)
    return in_maps, meta


# --------------------------------------------------------------------------
# Bass program
# --------------------------------------------------------------------------

def _build(meta):
    import concourse.bass as bass
    import concourse.mybir as mybir
    import concourse.tile as tile
    from concourse import bacc
    from concourse.masks import make_identity

    dt = mybir.dt
    ALU = mybir.AluOpType
    ACT = mybir.ActivationFunctionType
    DR = mybir.MatmulPerfMode.DoubleRow

    N, Nc, NG, FT, MT = meta["N"], meta["Nc"], meta["NG"], meta["FT"], meta["MT"]
    HID, MID, NCLS = meta["HID"], meta["MID"], meta["NCLS"]
    K, EG, IDXW, G = meta["K"], meta["EG"], meta["IDXW"], meta["G"]
    Kg = meta["Kg"]
    FT2 = FT // 2
    NCH = Nc // 512                    # 512-node chunks (= 4)
    QF = HID // 512                    # 512-wide feature quarters

    NOCC = bool(os.environ.get("GCN_NOCC"))
    rg = [list(range(C))]

    nc = bacc.Bacc("TRN2", target_bir_lowering=False, debug=False,
                   num_devices=1 if NOCC else C)

    def collective(kind, op, ins, outs):
        if NOCC:
            iap, oap = ins[0], outs[0]
            if kind == "AllGather":
                nc.sync.dma_start(oap[:iap.shape[0]], iap)
            else:
                nc.sync.dma_start(oap, iap)
        else:
            nc.gpsimd.collective_compute(kind, op, replica_groups=rg,
                                         ins=[ins[0].opt()], outs=[outs[0].opt()])

    # ---- inputs
    t_h128 = nc.dram_tensor("h128", [N, P], dt.bfloat16, kind="ExternalInput")
    t_hT = nc.dram_tensor("hT", [64, Nc], dt.bfloat16, kind="ExternalInput")
    t_gidx = nc.dram_tensor("gidx", [P, NG, IDXW], dt.int16, kind="ExternalInput")
    t_s1 = nc.dram_tensor("s1", [P, NG, K, P], dt.bfloat16, kind="ExternalInput")
    t_stc = nc.dram_tensor("stc", [P, NG, K, P], dt.float8e4,
                           kind="ExternalInput")
    t_w1s = nc.dram_tensor("w1s", [64, FT, P], dt.bfloat16, kind="ExternalInput")
    t_w1n = nc.dram_tensor("w1n", [64, FT, P], dt.bfloat16, kind="ExternalInput")
    t_w8 = {}
    for nm in ("w2s", "w3s"):
        t_w8[nm] = nc.dram_tensor(nm, [P, FT, FT2, 2, 2, P], dt.float8e4,
                                  kind="ExternalInput")
    for nm in ("w2n", "w3n"):
        t_w8[nm] = nc.dram_tensor(nm, [P, FT, FT2, 2, P], dt.float8e4,
                                  kind="ExternalInput")
    t_wf1 = nc.dram_tensor("wf1", [P, FT, FT, P], dt.bfloat16, kind="ExternalInput")
    t_wf2 = nc.dram_tensor("wf2", [P, MT, FT, P], dt.bfloat16, kind="ExternalInput")
    t_wf3 = nc.dram_tensor("wf3", [P, MT, NCLS], dt.bfloat16, kind="ExternalInput")
    t_bn = {}
    for nm in ("bn1g", "bn1b", "bn2g", "bn2b", "bn3g", "bn3b", "f1b"):
        t_bn[nm] = nc.dram_tensor(nm, [P, FT], dt.float32, kind="ExternalInput")
    t_bn["f2b"] = nc.dram_tensor("f2b", [P, MT], dt.float32, kind="ExternalInput")
    t_f3b = nc.dram_tensor("f3b", [NCLS, 1], dt.float32, kind="ExternalInput")
    t_pmat = nc.dram_tensor("pmat", [P, NG, G], dt.float8e4, kind="ExternalInput")
    t_invcnt = nc.dram_tensor("invcnt", [G, 1], dt.float32, kind="ExternalInput")
    t_out = nc.dram_tensor("out", [G, NCLS], dt.float32, kind="ExternalOutput")
    t_chain = nc.dram_tensor("chain", [G, NCLS], dt.float32, kind="ExternalInput")

    with tile.TileContext(nc) as tc:
        import contextlib
        ctx = contextlib.ExitStack()
        with ctx:
            dram = ctx.enter_context(tc.tile_pool(name="dram", bufs=1, space="DRAM"))
            consts = ctx.enter_context(tc.tile_pool(name="consts", bufs=1))
            work = ctx.enter_context(tc.tile_pool(name="work", bufs=1))
            psp = ctx.enter_context(tc.tile_pool(name="psp", bufs=8, space="PSUM"))

            # ---- DRAM scratch
            ynm8 = dram.tile([Nc, HID], dt.float8e4)
            # NOCC: bn writes its shard of yfull directly (no gather-copy);
            # real path: local ynm8 + AllGather collective into Shared yfull.
            yfull = [dram.tile([N, HID], dt.float8e4,
                               addr_space="Local" if NOCC else "Shared",
                               name=f"yfull{i}") for i in range(2)]
            stat_in = [dram.tile([P, 2 * FT], dt.float32, name=f"sti{i}")
                       for i in range(3)]
            stat_out = [dram.tile([P, 2 * FT], dt.float32, addr_space="Shared",
                                  name=f"sto{i}") for i in range(3)]
            pool_in = dram.tile([G, HID], dt.bfloat16)
            pool_out = dram.tile([G, HID], dt.bfloat16, addr_space="Shared")

            # ---- constants to SBUF
            idx_t = consts.tile([P, NG, IDXW], dt.int16)
            nc.sync.dma_start(idx_t[:], t_gidx[:])
            hT_t = consts.tile([64, Nc], dt.bfloat16)
            nc.sync.dma_start(hT_t[:], t_hT[:])
            w1s_t = consts.tile([64, FT, P], dt.bfloat16)
            nc.sync.dma_start(w1s_t[:], t_w1s[:])
            w1n_t = consts.tile([64, FT, P], dt.bfloat16)
            nc.sync.dma_start(w1n_t[:], t_w1n[:])
            pmat_t = consts.tile([P, NG, G], dt.float8e4)
            nc.sync.dma_start(pmat_t[:], t_pmat[:])
            invcnt_t = consts.tile([G, 1], dt.float32)
            nc.sync.dma_start(invcnt_t[:], t_invcnt[:])
            wf3_t = consts.tile([P, MT, NCLS], dt.bfloat16)
            nc.sync.dma_start(wf3_t[:], t_wf3[:])
            f3b_t = consts.tile([NCLS, 1], dt.float32)
            nc.sync.dma_start(f3b_t[:], t_f3b[:])
            bn_t = {}
            for nm, th in t_bn.items():
                bn_t[nm] = consts.tile(list(th.shape), dt.float32, name=f"c_{nm}")
                nc.sync.dma_start(bn_t[nm][:], th[:])
            ident_f8 = consts.tile([P, P], dt.float8e4)
            make_identity(nc, ident_f8[:])
            ident_bf = consts.tile([P, P], dt.bfloat16)
            make_identity(nc, ident_bf[:])
            ident_f32 = consts.tile([P, P], dt.float32)
            make_identity(nc, ident_f32[:])

            # ---- persistent activations (SBUF-resident, reused per layer);
            # tagged so the head can reuse their space for weight staging
            # once they die (m8 after dense3, rst/y8 after bn3)
            rst = work.tile([P, FT, Nc], dt.bfloat16, tag="rsts", bufs=1)
            y8 = work.tile([P, FT, Nc], dt.float8e4, tag="y8s", bufs=1)
            m8 = work.tile([P, FT, Nc], dt.float8e4, tag="m8s", bufs=1)
            m1T = work.tile([64, Nc], dt.bfloat16, tag="m1s", bufs=1)
            st6 = work.tile([P, FT, NCH, 6], dt.float32)
            mv2 = work.tile([P, FT, 2], dt.float32)

            # ---------------- helpers ----------------
            def layer1():
                """agg quad (4 groups -> m1T cols) then dense_l1 on that
                node chunk, interleaved for engine overlap."""
                for g4 in range(NG // 4):
                    ps = psp.tile([P, 512], dt.float32, tag="ps",
                                  name=f"aps1_{g4}")
                    for gi in range(4):
                        g = g4 * 4 + gi
                        KG = Kg[g]
                        S_g = work.tile([P, K, P], dt.bfloat16, tag="sg",
                                        bufs=2, name=f"sg1_{g}")
                        nc.sync.dma_start(S_g[:, :KG, :], t_s1[:, g, :KG, :])
                        Gt = work.tile([P, K, P], dt.bfloat16, tag="g1",
                                       bufs=2, name=f"G1_{g}")
                        nc.gpsimd.dma_gather(
                            out_ap=Gt[:], in_ap=t_h128[:],
                            idxs_ap=idx_t[:, g, :KG * 8],
                            num_idxs=KG * P, num_idxs_reg=KG * P,
                            elem_size=P)
                        for k in range(KG):
                            nc.tensor.matmul(ps[:64, gi * P:(gi + 1) * P],
                                             lhsT=Gt[:, k, :64],
                                             rhs=S_g[:, k, :],
                                             start=(k == 0), stop=(k == KG - 1))
                    nc.vector.tensor_copy(m1T[:, g4 * 512:(g4 + 1) * 512],
                                          ps[:64, :])
                    ch = g4
                    sl = slice(ch * 512, ch * 512 + 512)
                    for fo in range(FT):
                        dps = psp.tile([P, 512], dt.float32, tag="ps",
                                       name=f"dps1_{fo}_{ch}")
                        nc.tensor.matmul(dps[:], lhsT=w1s_t[:, fo, :],
                                         rhs=hT_t[:, sl], start=True, stop=False)
                        nc.tensor.matmul(dps[:], lhsT=w1n_t[:, fo, :],
                                         rhs=m1T[:, sl], start=False, stop=True)
                        nc.vector.bn_stats(st6[:, fo, ch, :], dps[:])
                        nc.scalar.activation(rst[:, fo, sl], dps[:], ACT.Copy)
                for fo in range(FT):
                    nc.vector.bn_aggr(mv2[:, fo, :], st6[:, fo, :, :])

            def agg_fetch(li, g):
                """Issue the gather + S-matrix loads for group g."""
                ysrc = yfull[li - 2]
                KG = Kg[g]
                Gt = work.tile([P, K, HID], dt.float8e4, tag="big", bufs=2,
                               name=f"G{li}_{g}")
                nc.gpsimd.dma_gather(
                    out_ap=Gt[:], in_ap=ysrc[:],
                    idxs_ap=idx_t[:, g, :KG * 8],
                    num_idxs=KG * P, num_idxs_reg=KG * P,
                    elem_size=HID)
                st_g = work.tile([P, K, P], dt.float8e4, tag="sg",
                                 bufs=2, name=f"st{li}_{g}")
                nc.sync.dma_start(st_g[:, :KG, :], t_stc[:, g, :KG, :])
                return Gt, st_g

            def agg_mm(li, g, Gt, st_g):
                """m8 columns of group g from its gathered rows (fp8 DR,
                invdeg folded into S, one PSUM group per bank)."""
                KG = Kg[g]
                K2 = KG // 2
                gsl = slice(g * P, (g + 1) * P)
                nmm = (K2 + (1 if KG % 2 else 0))
                for fq in range(4):
                    ps = psp.tile([P, 512], dt.float32, tag="ps",
                                  name=f"aps{li}_{g}_{fq}")
                    tot = 4 * nmm
                    i = 0
                    for fi in range(4):
                        ft = fq * 4 + fi
                        fsl = slice(ft * P, (ft + 1) * P)
                        osl = slice(fi * P, (fi + 1) * P)
                        for k2 in range(K2):
                            nc.tensor.matmul(
                                ps[:, osl],
                                lhsT=Gt[:, 2 * k2:2 * k2 + 2, fsl],
                                rhs=st_g[:, 2 * k2:2 * k2 + 2, :],
                                start=(i == 0), stop=(i == tot - 1),
                                perf_mode=DR, skip_group_check=True)
                            i += 1
                        if KG % 2:
                            nc.tensor.matmul(
                                ps[:, osl], lhsT=Gt[:, KG - 1, fsl],
                                rhs=st_g[:, KG - 1, :],
                                start=(i == 0), stop=(i == tot - 1),
                                skip_group_check=True)
                            i += 1
                    ps4 = ps.rearrange("p (a b) -> p a b", a=4)
                    if (g + fq) % 2 == 0:
                        nc.vector.tensor_copy(
                            m8[:, 4 * fq:4 * fq + 4, gsl], ps4[:])
                    else:
                        nc.scalar.copy(m8[:, 4 * fq:4 * fq + 4, gsl], ps4[:])

            def agg(li):
                """First half of the groups: fetch + matmul. The second half
                is interleaved into dense() pass A."""
                for g in range(8):
                    Gt, st_g = agg_fetch(li, g)
                    agg_mm(li, g, Gt, st_g)

            def dense(li):
                """Two chunk-half passes; groups 8-15 of the aggregation are
                fetched and matmul'd between pass-A fo iterations (their m8
                columns are only needed by pass B)."""
                ws_d = t_w8[f"w{li}s"]
                wn_d = t_w8[f"w{li}n"]
                pend = None
                for ph in range(2):
                    for fo in range(FT):
                        wsc = work.tile([P, FT2, 2, 2, P], dt.float8e4,
                                        tag="wcol", bufs=3,
                                        name=f"ws{li}_{ph}_{fo}")
                        nc.sync.dma_start(wsc[:], ws_d[:, fo])
                        wnc = work.tile([P, FT2, 2, P], dt.float8e4,
                                        tag="wcol", bufs=3,
                                        name=f"wn{li}_{ph}_{fo}")
                        nc.sync.dma_start(wnc[:], wn_d[:, fo])
                        for ch in (2 * ph, 2 * ph + 1):
                            sl = slice(ch * 512, ch * 512 + 512)
                            ps = psp.tile([P, 512], dt.float32, tag="ps",
                                          name=f"dps{li}_{fo}_{ch}")
                            first = True
                            for kt2 in range(FT2):
                                for hl in range(2):
                                    nc.tensor.matmul(
                                        ps[:], lhsT=wsc[:, kt2, hl],
                                        rhs=y8[:, 2 * kt2:2 * kt2 + 2, sl],
                                        start=first, stop=False,
                                        perf_mode=DR)
                                    first = False
                            for kt2 in range(FT2):
                                nc.tensor.matmul(
                                    ps[:], lhsT=wnc[:, kt2],
                                    rhs=m8[:, 2 * kt2:2 * kt2 + 2, sl],
                                    start=False, stop=(kt2 == FT2 - 1),
                                    perf_mode=DR)
                            nc.vector.bn_stats(st6[:, fo, ch, :], ps[:])
                            nc.scalar.activation(rst[:, fo, sl], ps[:],
                                                 ACT.Copy)
                        if ph == 0 and fo % 2 == 0:
                            gi = 8 + fo // 2
                            nxt = agg_fetch(li, gi)
                            if pend is not None:
                                agg_mm(li, gi - 1, *pend)
                            pend = nxt
                        if ph == 1:
                            nc.vector.bn_aggr(mv2[:, fo, :],
                                              st6[:, fo, :, :])
                    if ph == 0:
                        agg_mm(li, 15, *pend)

            def stats(li):
                """bn_aggr + AllReduce(sum, sumsq) -> a, b affine coeffs."""
                acc = work.tile([P, 2 * FT], dt.float32, tag="acc", bufs=2,
                                name=f"acc{li}")
                # sum = mean * Nc
                nc.vector.tensor_scalar(acc[:, :FT], mv2[:, :, 0], float(Nc),
                                        None, ALU.mult)
                # sumsq = (var + mean^2) * Nc
                tmp = work.tile([P, FT], dt.float32, tag="acc2", bufs=4,
                                name=f"tmp{li}")
                nc.vector.tensor_tensor(tmp[:], mv2[:, :, 0], mv2[:, :, 0],
                                        ALU.mult)
                nc.vector.tensor_tensor(tmp[:], tmp[:], mv2[:, :, 1], ALU.add)
                nc.vector.tensor_scalar(acc[:, FT:], tmp[:], float(Nc),
                                        None, ALU.mult)
                if NOCC:
                    sums = acc          # single-core AllReduce is identity
                else:
                    nc.sync.dma_start(stat_in[li - 1][:], acc[:])
                    collective("AllReduce", ALU.add, [stat_in[li - 1]],
                               [stat_out[li - 1]])
                    sums = work.tile([P, 2 * FT], dt.float32, tag="sums",
                                     bufs=2, name=f"sm{li}")
                    nc.sync.dma_start(sums[:], stat_out[li - 1][:])
                mu = work.tile([P, FT], dt.float32, tag="acc2", bufs=4,
                               name=f"mu{li}")
                nc.vector.tensor_scalar(mu[:], sums[:, :FT], 1.0 / N, None,
                                        ALU.mult)
                var = work.tile([P, FT], dt.float32, tag="acc2", bufs=4,
                                name=f"vr{li}")
                nc.vector.tensor_scalar(var[:], sums[:, FT:], 1.0 / N, EPS,
                                        ALU.mult, ALU.add)
                tmq = work.tile([P, FT], dt.float32, tag="acc2", bufs=4,
                                name=f"tq{li}")
                nc.vector.tensor_tensor(tmq[:], mu[:], mu[:], ALU.mult)
                nc.vector.tensor_tensor(var[:], var[:], tmq[:], ALU.subtract)
                std = work.tile([P, FT], dt.float32, tag="acc2", bufs=4,
                                name=f"sd{li}")
                nc.scalar.activation(std[:], var[:], ACT.Sqrt)
                rstd = work.tile([P, FT], dt.float32, tag="acc2", bufs=4,
                                 name=f"rs{li}")
                nc.vector.reciprocal(rstd[:], std[:])
                a_sb = work.tile([P, FT], dt.float32, tag="ab", bufs=2,
                                 name=f"a{li}")
                b_sb = work.tile([P, FT], dt.float32, tag="ab", bufs=2,
                                 name=f"b{li}")
                nc.vector.tensor_tensor(a_sb[:], rstd[:], bn_t[f"bn{li}g"][:],
                                        ALU.mult)
                tmp2 = work.tile([P, FT], dt.float32, tag="acc2", bufs=4,
                                 name=f"tm2{li}")
                nc.vector.tensor_tensor(tmp2[:], mu[:], a_sb[:], ALU.mult)
                nc.vector.tensor_tensor(b_sb[:], bn_t[f"bn{li}b"][:], tmp2[:],
                                        ALU.subtract)
                return a_sb, b_sb

            def bn_apply(li, a_sb, b_sb, pool_ps):
                """y8 = lrelu(a*rst+b) (fp8); transpose to node-major; DMA to
                ynm8 (li<3) or pool-matmul (li==3)."""
                for qq in range(NCH):
                    q0 = qq * 512
                    for ft in range(FT):
                        nc.scalar.activation(
                            y8[:, ft, q0:q0 + 512], rst[:, ft, q0:q0 + 512],
                            ACT.Lrelu, bias=b_sb[:, ft:ft + 1],
                            scale=a_sb[:, ft:ft + 1], alpha=SLOPE)
                    for j in range(4):
                        nt = qq * 4 + j
                        n0 = q0 + j * P
                        yT = work.tile([P, HID], dt.float8e4, tag="yT",
                                       bufs=2, name=f"yT{li}_{nt}")
                        for fh in range(2):
                            tp = psp.tile([P, 512], dt.float32, tag="ps",
                                          name=f"ytp{li}_{nt}_{fh}")
                            tp8 = tp.bitcast(dt.float8e4)
                            for fi in range(8):
                                ft = fh * 8 + fi
                                nc.tensor.transpose(
                                    tp8[:, fi * 256:fi * 256 + 2 * P:2],
                                    y8[:, ft, n0:n0 + P], ident_f8[:])
                            nc.vector.tensor_copy(
                                yT[:, fh * 1024:(fh + 1) * 1024],
                                tp8[:, 0:2048:2])
                        if li < 3:
                            if NOCC:
                                nc.sync.dma_start(
                                    yfull[li - 1][n0:n0 + P, :], yT[:])
                            else:
                                nc.sync.dma_start(ynm8[n0:n0 + P, :], yT[:])
                        else:
                            for q in range(QF):
                                nc.tensor.matmul(
                                    pool_ps[q][:G],
                                    lhsT=pmat_t[:, nt, :],
                                    rhs=yT[:, q * 512:(q + 1) * 512],
                                    start=(nt == 0), stop=(nt == NG - 1),
                                    skip_group_check=True)

            # ---------------- the network ----------------
            STAGE = os.environ.get("GCN_STAGE", "full")
            pool_ps = None
            done = False
            for li in (1, 2, 3):
                if li == 1:
                    layer1()
                    if STAGE == "agg1":
                        done = True
                        break
                else:
                    agg(li)
                    if STAGE == f"agg{li}":
                        done = True
                        break
                    dense(li)
                if STAGE == f"dense{li}":
                    done = True
                    break
                a_sb, b_sb = stats(li)
                if STAGE == f"stats{li}":
                    done = True
                    break
                if li == 3:
                    pool_ps = [psp.tile([P, 512], dt.float32, tag="ps",
                                        name=f"pps{q}") for q in range(QF)]
                    # stage fc1 cols 0-7 in m8's slot (dead after dense3,
                    # load overlaps stats3 + bn3); cols 12-14 in the wcol
                    # slots (dead after dense3); col 15 in m1T's slot
                    wf1a = work.tile([P, 8, FT, P], dt.bfloat16, tag="m8s",
                                     bufs=1, name="wf1a")
                    nc.sync.dma_start(wf1a[:], t_wf1[:, :8])
                    wf1c = [work.tile([P, 2, FT, P], dt.bfloat16,
                                      tag="big", bufs=2, name=f"wf1c{i}")
                            for i in range(2)]
                    for i in range(2):
                        nc.sync.dma_start(wf1c[i][:],
                                          t_wf1[:, 8 + 2 * i:10 + 2 * i])
                    wf1w = [work.tile([P, 1, FT, P], dt.bfloat16, tag="wcol",
                                      bufs=3, name=f"wf1w{i}")
                            for i in range(3)]
                    for i in range(3):
                        nc.sync.dma_start(wf1w[i][:], t_wf1[:, 12 + i:13 + i])
                    wf1m = work.tile([P, 1, FT, P], dt.bfloat16, tag="m1s",
                                     bufs=1, name="wf1m")
                    nc.sync.dma_start(wf1m[:], t_wf1[:, 15:16])
                bn_apply(li, a_sb, b_sb, pool_ps)
                if li == 3 and STAGE == "full":
                    # fc2 in y8's slot (dead at the end of bn3) - the load
                    # overlaps the pool chain and fc1 compute
                    wf2a = work.tile([P, MT, FT, P], dt.bfloat16, tag="y8s",
                                     bufs=1, name="wf2a")
                    nc.sync.dma_start(wf2a[:], t_wf2[:])
                if STAGE == f"bn{li}":
                    done = True
                    break
                if li < 3 and not NOCC:
                    collective("AllGather", ALU.bypass, [ynm8], [yfull[li - 1]])
                if STAGE == f"ag{li}":
                    done = True
                    break

            if done:
                dbg = work.tile([G, NCLS], dt.float32, tag="dbg", bufs=1)
                nc.vector.tensor_copy(dbg[:], rst.bitcast(dt.float32)[:G, 0, :NCLS])
                nc.sync.dma_start(t_out[:], dbg[:])
            else:
                # ---------------- pooling + head ----------------
                hgsb = work.tile([G, HID], dt.bfloat16, tag="hg", bufs=2,
                                 name="hgsb")
                for q in range(QF):
                    nc.vector.tensor_copy(hgsb[:, q * 512:(q + 1) * 512],
                                          pool_ps[q][:G])
                hgr = work.tile([G, HID], dt.bfloat16, tag="hg", bufs=2,
                                name="hgr")
                if NOCC:
                    nc.vector.tensor_scalar(hgr[:], hgsb[:], invcnt_t[:, 0:1],
                                            None, ALU.mult)
                else:
                    nc.sync.dma_start(pool_in[:], hgsb[:])
                    collective("AllReduce", ALU.add, [pool_in], [pool_out])
                    hgr0 = work.tile([G, HID], dt.bfloat16, tag="hg", bufs=2,
                                     name="hgr0")
                    nc.sync.dma_start(hgr0[:], pool_out[:])
                    nc.vector.tensor_scalar(hgr[:], hgr0[:], invcnt_t[:, 0:1],
                                            None, ALU.mult)
                hg_fm = work.tile([P, FT, G], dt.bfloat16, tag="hgfm", bufs=1)
                for ft in range(FT):
                    tp = psp.tile([P, 512], dt.float32, tag="ps",
                                  name=f"htp{ft}")
                    tpbf = tp.bitcast(dt.bfloat16)
                    nc.tensor.transpose(tpbf[:, :G], hgr[:, ft * P:(ft + 1) * P],
                                        ident_bf[:G, :G])
                    nc.vector.tensor_copy(hg_fm[:, ft, :], tpbf[:, :G])

                def fc_layer(wtiles, kt_count, fo_count, xin, bias_t, name):
                    xout = work.tile([P, fo_count, G], dt.bfloat16,
                                     tag=f"x{name}", bufs=1, name=f"x{name}")
                    for fo in range(fo_count):
                        wt, wi = wtiles[fo]
                        ps = psp.tile([P, 512], dt.float32, tag="ps",
                                      name=f"hps{name}_{fo}")
                        for k in range(kt_count):
                            nc.tensor.matmul(ps[:, :G],
                                             lhsT=wt[:, wi, k, :],
                                             rhs=xin[:, k, :], start=(k == 0),
                                             stop=(k == kt_count - 1))
                        nc.scalar.activation(xout[:, fo, :], ps[:, :G],
                                             ACT.Lrelu,
                                             bias=bias_t[:, fo:fo + 1],
                                             scale=1.0, alpha=SLOPE)
                    return xout

                f1_tiles = ([(wf1a, i) for i in range(8)]
                            + [(wf1c[0], 0), (wf1c[0], 1),
                               (wf1c[1], 0), (wf1c[1], 1)]
                            + [(wf1w[i], 0) for i in range(3)]
                            + [(wf1m, 0)])
                x1 = fc_layer(f1_tiles, FT, FT, hg_fm, bn_t["f1b"], "f1")
                x2 = fc_layer([(wf2a, i) for i in range(MT)], FT, MT, x1,
                              bn_t["f2b"], "f2")

                ps18 = psp.tile([P, 512], dt.float32, tag="ps", name="ps18")
                for k in range(MT):
                    nc.tensor.matmul(ps18[:NCLS, :G], lhsT=wf3_t[:, k, :],
                                     rhs=x2[:, k, :], start=(k == 0),
                                     stop=(k == MT - 1))
                o18 = work.tile([NCLS, G], dt.float32, tag="o18", bufs=1)
                nc.vector.tensor_scalar(o18[:], ps18[:NCLS, :G], f3b_t[:, 0:1],
                                        None, ALU.add)
                tp = psp.tile([P, 512], dt.float32, tag="ps", name="otp")
                nc.tensor.transpose(tp[:G, :NCLS], o18[:],
                                    ident_f32[:NCLS, :NCLS])
                osb = work.tile([G, NCLS], dt.float32, tag="osb", bufs=1)
                nc.vector.tensor_copy(osb[:], tp[:G, :NCLS])
                chn = work.tile([G, NCLS], dt.float32, tag="chn", bufs=1)
                nc.sync.dma_start(chn[:], t_chain[:])
                nc.vector.tensor_scalar(chn[:], chn[:], 0.0, None, ALU.mult)
                nc.vector.tensor_tensor(osb[:], osb[:], chn[:], ALU.add)
                nc.sync.dma_start(t_out[:], osb[:])

    nc.compile()
    return nc


# --------------------------------------------------------------------------
# entry point
# --------------------------------------------------------------------------

LAST_EXEC_NS = None
LAST_TRACE = None


def _run_timed(nc, in_maps, iters=4, reps=None):
    """Mirror bass2jax.run_bass_via_pjrt but keep inputs device-resident so
    warm re-executions measure the on-device program span."""
    import time
    import jax
    from jax.sharding import Mesh, PartitionSpec
    from jax.experimental.shard_map import shard_map
    import concourse.mybir as mybir
    from concourse.bass2jax import (
        install_neuronx_cc_hook, _bass_exec_p, partition_id_tensor)

    install_neuronx_cc_hook()
    n_cores = len(in_maps)
    partition_name = nc.partition_id_tensor.name if nc.partition_id_tensor else None
    in_names, out_names, out_avals, zero_outs = [], [], [], []
    for alloc in nc.m.functions[0].allocations:
        if not isinstance(alloc, mybir.MemoryLocationSet):
            continue
        name = alloc.memorylocations[0].name
        if alloc.kind == "ExternalInput":
            if name != partition_name:
                in_names.append(name)
        elif alloc.kind == "ExternalOutput":
            shape = tuple(alloc.tensor_shape)
            dtype = mybir.dt.np(alloc.dtype)
            out_names.append(name)
            out_avals.append(jax.core.ShapedArray(shape, dtype))
            zero_outs.append(np.zeros((n_cores * shape[0], *shape[1:]), dtype))
    n_params = len(in_names)
    all_in = list(in_names) + list(out_names)
    if partition_name is not None:
        all_in.append(partition_name)

    if reps is None:
        reps = int(os.environ.get("GCN_REPS", "1"))

    chain_idx = in_names.index("chain") if "chain" in in_names else None
    out_idx = out_names.index("out") if "out" in out_names else None

    def _body(*args):
        operands = list(args)
        if partition_name is not None:
            operands.append(partition_id_tensor())
        for _ in range(reps):
            outs = _bass_exec_p.bind(
                *operands, out_avals=tuple(out_avals), in_names=tuple(all_in),
                out_names=tuple(out_names), lowering_input_output_aliases=(),
                sim_require_finite=True, sim_require_nnan=True, nc=nc)
            if chain_idx is not None and out_idx is not None:
                operands[chain_idx] = outs[out_idx]
        return tuple(outs)

    devices = jax.devices()[:n_cores]
    mesh = Mesh(np.asarray(devices), ("core",))
    nin = n_params + len(out_names)
    sharded = jax.jit(
        shard_map(_body, mesh=mesh, in_specs=(PartitionSpec("core"),) * nin,
                  out_specs=(PartitionSpec("core"),) * len(out_names),
                  check_rep=False),
        donate_argnums=tuple(range(n_params, nin)), keep_unused=True)

    shd = jax.sharding.NamedSharding(mesh, PartitionSpec("core"))
    dev_in = [
        jax.device_put(
            np.concatenate([np.asarray(in_maps[c][nm]) for c in range(n_cores)],
                           axis=0), shd)
        for nm in in_names
    ]
    times = []
    outs = None
    for _ in range(iters):
        zo = [jax.device_put(z.copy(), shd) for z in zero_outs]
        for z in zo:
            z.block_until_ready()
        t0 = time.perf_counter()
        outs = sharded(*dev_in, *zo)
        for o in outs:
            o.block_until_ready()
        times.append(time.perf_counter() - t0)
    best_ns = int(min(times) * 1e9 / reps)
    results = [
        {nm: np.asarray(outs[i]).reshape(n_cores, *out_avals[i].shape)[c]
         for i, nm in enumerate(out_names)}
        for c in range(n_cores)
    ]
    print(f"timed runs (s, reps={reps}): {[f'{t:.4f}' for t in times]}")
    return results, best_ns


def kernel(**inputs) -> np.ndarray:
    global LAST_EXEC_NS, LAST_TRACE
    from concourse.bass_utils import run_bass_kernel_spmd

    in_maps, meta = _preprocess(inputs)
    nc = _build(meta)
    in_maps = [{k: np.ascontiguousarray(v) for k, v in m.items()}
               for m in in_maps]
    if os.environ.get("GCN_TIME"):
        results, best_ns = _run_timed(nc, in_maps)
        LAST_EXEC_NS = best_ns
        return np.asarray(results[0]["out"], np.float32)
    res = run_bass_kernel_spmd(nc, in_maps, core_ids=list(range(C)))
    LAST_EXEC_NS = res.exec_time_ns
    LAST_TRACE = res.instructions_and_trace
    return np.asarray(res.results[0]["out"], np.float32)
